# revision 20
# baseline (speedup 1.0000x reference)
"""Trainium2 Bass kernel for nn_BaseNCA (NCA: 3x3 Sobel + per-pixel MLP, 4 steps).

Sharding: pure data parallel over 8 cores = (batch b, H-half). Each core gets one
batch's top or bottom half of H (128 rows) plus a 4-row halo toward the middle
(1 conv ring per step x 4 steps). No collectives: validity shrinks one row per
step into the halo; the kept 128-row window is exact after 4 steps. The tile
edge that is a true image boundary zero-pads identically to the reference.

Per-core math folding (host side):
  FiLM gamma/beta are step-invariant; with g = gamma, a=|g|, s=sign(g):
    g*relu(p + b1) + beta == s*relu(a*p + a*b1) + beta
  so scale fc1 columns by a, fold s into fc2 rows and beta@fc2_w into the fc2
  bias. The Sobel convs are linear, so fc1 on [x, gx, gy] becomes 9 shifted
  16->128 matmuls with effective kernels Keff[di][dj]; the dx scale 0.1 is
  folded into fc3 (clip bounds become +-1).

Device layout: state [128 partitions = (c + 16*(j%8)), free = (row, t=j//8)]
with one zero pad column on each side of the 32 t-slots (row stride 34) and one
zero guard row above/below the 132 rows. Conv taps dj=-1,0,+1 then live in
adjacent 16-partition blocks, so fc1 is 3 accumulated K=48 matmuls (one per
di, free-offset +-1 row); W-wraparound classes 0/7 split into K=32 + K=16.
fc3 writes dx for all 8 classes of a row block into one PSUM [128, nr*32]
(8 accumulated M=128 matmuls with host-expanded weights), so the clip +
state-update runs on all 128 lanes. Matmuls use float32r (full speed at
N>=256, ~tf32 precision, fp32 accumulate); state stays fp32.
"""

import sys

import numpy as np

sys.path.insert(0, "/opt/trn_rl_repo")

import concourse.bass as bass
import concourse.mybir as mybir
from concourse.bacc import Bacc
from concourse.bass_utils import run_bass_kernel_spmd
from concourse.tile import TileContext

C, HID, W = 16, 128, 256
HE = 132  # extended rows per core (128 kept + 4 halo toward the middle)
T = W // 8  # 32 t-slots per class
RS = T + 2  # row stride incl. one pad col each side
NR_TOT = 1 + HE + 1  # incl. zero guard rows
SX = np.array([[-1.0, 0.0, 1.0], [-2.0, 0.0, 2.0], [-1.0, 0.0, 1.0]], np.float64)
SY = SX.T

ROW_BLOCKS = [(i, min(16, HE - i)) for i in range(0, HE, 16)]  # 8x16 + 1x4

# fc1 matmul plan: for each output class r and row shift di, a list of
# (kind_index, t_offset) matmuls. Each kind is a [128,128] lhsT with the
# Keff blocks embedded at the partition rows of the source classes; rhs is
# always the full 128-partition state (base partition 0, as HW requires).
MM_PLAN = {}
MM_KINDS = []  # list of (di, [(dj, src_class), ...])
for _r in range(8):
    for _di in (-1, 0, 1):
        ent = []
        if 1 <= _r <= 6:
            MM_KINDS.append((_di, [(dj, _r + dj) for dj in (-1, 0, 1)]))
            ent.append((len(MM_KINDS) - 1, 0))
        elif _r == 0:
            MM_KINDS.append((_di, [(0, 0), (1, 1)]))
            ent.append((len(MM_KINDS) - 1, 0))
            MM_KINDS.append((_di, [(-1, 7)]))
            ent.append((len(MM_KINDS) - 1, -1))
        else:
            MM_KINDS.append((_di, [(-1, 6), (0, 7)]))
            ent.append((len(MM_KINDS) - 1, 0))
            MM_KINDS.append((_di, [(1, 0)]))
            ent.append((len(MM_KINDS) - 1, 1))
        MM_PLAN[(_r, _di)] = ent
N_KINDS = len(MM_KINDS)


def fold_weights(gamma, beta, fc1_w, fc1_b, fc2_w, fc2_b, fc3_w, fc3_b):
    """Per-batch folded weights (float64 in, device arrays out)."""
    a = np.abs(gamma)
    s = np.sign(gamma)
    W1x, W1gx, W1gy = fc1_w[0:16], fc1_w[16:32], fc1_w[32:48]

    def keff(di, dj):
        k = SX[di + 1, dj + 1] * W1gx + SY[di + 1, dj + 1] * W1gy
        if di == 0 and dj == 0:
            k = k + W1x
        return k * a[None, :]

    w1 = np.zeros((N_KINDS, 128, 128), np.float64)
    for idx, (di, blocks) in enumerate(MM_KINDS):
        for dj, cls in blocks:
            w1[idx, 16 * cls : 16 * cls + 16, :] = keff(di, dj)
    b1 = a * fc1_b
    w2 = s[:, None] * fc2_w
    b2 = beta @ fc2_w + fc2_b
    w3big = np.zeros((128, 8, 128), np.float64)
    for r in range(8):
        w3big[:, r, 16 * r : 16 * r + 16] = 0.1 * fc3_w
    b3t = np.tile(0.1 * fc3_b, 8)
    f32 = np.float32
    return {
        "w1": np.ascontiguousarray(w1.transpose(1, 0, 2).reshape(128, N_KINDS * 128)).astype(f32),
        "w2": np.ascontiguousarray(w2).astype(f32),
        "w3": np.ascontiguousarray(w3big.reshape(128, 8 * 128)).astype(f32),
        "bb": np.stack([b1, b2, b3t], axis=1).astype(f32),
    }


def shuffle_in(x_ext):
    """[16, 132, 256] -> [128, NR_TOT*RS] blocked layout with zero pads/guards."""
    xb = np.zeros((8, 16, NR_TOT, RS), np.float32)
    for r in range(8):
        xb[r, :, 1 : 1 + HE, 1 : 1 + T] = x_ext[:, :, r::8]
    return xb.reshape(128, NR_TOT * RS)


def unshuffle_out(res):
    """[128, HE*RS] -> [16, 132, 256]."""
    rb = res.reshape(8, 16, HE, RS)
    y = np.empty((16, HE, W), np.float32)
    for r in range(8):
        y[:, :, r::8] = rb[r, :, :, 1 : 1 + T]
    return y


def build_graph(nc, n_steps):
    f32 = mybir.dt.float32
    f32r = mybir.dt.float32r
    relu = mybir.ActivationFunctionType.Relu
    add, mn, mx = mybir.AluOpType.add, mybir.AluOpType.min, mybir.AluOpType.max

    xin = nc.declare_dram_parameter("xb", [128, NR_TOT, RS], f32, isOutput=False)
    w1in = nc.declare_dram_parameter("w1", [128, N_KINDS * 128], f32, isOutput=False)
    w2in = nc.declare_dram_parameter("w2", [128, 128], f32, isOutput=False)
    w3in = nc.declare_dram_parameter("w3", [128, 8 * 128], f32, isOutput=False)
    bbin = nc.declare_dram_parameter("bb", [128, 3], f32, isOutput=False)
    outp = nc.declare_dram_parameter("out", [128, HE, RS], f32r, isOutput=True)

    with TileContext(nc) as tc:
        with (
            tc.tile_pool(name="const", bufs=1) as cpool,
            tc.tile_pool(name="work", bufs=3) as wpool,
            tc.tile_pool(name="ps", bufs=2, space="PSUM") as ppool,
        ):
            stA = cpool.tile([128, NR_TOT, RS], f32r, tag="stA")
            stB = cpool.tile([128, NR_TOT, RS], f32r, tag="stB")
            w1 = cpool.tile([128, N_KINDS * 128], f32r, tag="w1")
            w2 = cpool.tile([128, 128], f32r, tag="w2")
            w3 = cpool.tile([128, 8 * 128], f32r, tag="w3")
            bb = cpool.tile([128, 3], f32, tag="bb")

            # Bulk loads go DMA -> staging -> DVE copy so each matmul-feeding
            # tile has ONE producer proc (DVE); direct big DMAs fan out over
            # many DMA queues and blow the per-instruction sync-wait limit.
            stg_x = wpool.tile([128, NR_TOT, RS], f32, tag="stg_x")
            stg_w1 = wpool.tile([128, N_KINDS * 128], f32, tag="stg_w1")
            stg_w2 = wpool.tile([128, 128], f32, tag="stg_w2")
            stg_w3 = wpool.tile([128, 8 * 128], f32, tag="stg_w3")
            nc.sync.dma_start(out=stg_x[:, :, :], in_=xin[:, :, :])
            nc.sync.dma_start(out=stg_w1[:, :], in_=w1in[:, :])
            nc.sync.dma_start(out=stg_w2[:, :], in_=w2in[:, :])
            nc.sync.dma_start(out=stg_w3[:, :], in_=w3in[:, :])
            stg_b = wpool.tile([128, 3], f32, tag="stg_b")
            nc.sync.dma_start(out=stg_b[:, :], in_=bbin[:, :])
            nc.vector.tensor_copy(stA[:, :, :], stg_x[:, :, :])
            nc.vector.tensor_copy(stB[:, :, :], stg_x[:, :, :])
            nc.vector.tensor_copy(bb[:, :], stg_b[:, :])
            nc.vector.tensor_copy(w1[:, :], stg_w1[:, :])
            nc.vector.tensor_copy(w2[:, :], stg_w2[:, :])
            nc.vector.tensor_copy(w3[:, :], stg_w3[:, :])

            for step in range(n_steps):
                src, dst = (stA, stB) if step % 2 == 0 else (stB, stA)
                for i0, nr in ROW_BLOCKS:
                    ps3 = ppool.tile([128, 16, T], f32, tag="ps3")
                    for r in range(8):
                        ps1 = ppool.tile([128, 16, T], f32, tag="ps1")
                        # fc1: conv taps via full-K matmuls (zero-embedded lhsT
                        # kinds); rows 1+i0+di .. +nr; t-window shifts for the
                        # W-wraparound classes.
                        mms = []
                        for di in (-1, 0, 1):
                            for idx, toff in MM_PLAN[(r, di)]:
                                mms.append((idx, di, toff))
                        nmm = len(mms)
                        for q, (idx, di, toff) in enumerate(mms):
                            rsl = slice(1 + i0 + di, 1 + i0 + di + nr)
                            csl = slice(1 + toff, 1 + toff + T)
                            nc.tensor.matmul(
                                ps1[:, :nr, :],
                                w1[:, 128 * idx : 128 * (idx + 1)],
                                src[:, rsl, csl],
                                start=(q == 0),
                                stop=(q == nmm - 1),
                            )
                        h1 = wpool.tile([128, 16, T], f32r, tag="h1")
                        nc.scalar.activation(
                            h1[:, :nr, :], ps1[:, :nr, :], relu, bias=bb[:, 0:1], scale=1.0
                        )
                        ps2 = ppool.tile([128, 16, T], f32, tag="ps2")
                        nc.tensor.matmul(
                            ps2[:, :nr, :],
                            w2[:, :],
                            h1[:, :nr, :],
                            start=True,
                            stop=True,
                        )
                        h2 = wpool.tile([128, 16, T], f32r, tag="h2")
                        nc.vector.tensor_scalar(
                            h2[:, :nr, :], ps2[:, :nr, :], bb[:, 1:2], 0.0, add, mx
                        )
                        nc.tensor.matmul(
                            ps3[:, :nr, :],
                            w3[:, 128 * r : 128 * (r + 1)],
                            h2[:, :nr, :],
                            start=(r == 0),
                            stop=(r == 7),
                        )
                    # u = min(ps3 + b3, 1); dst = max(u, -1) + src  (clip +-1 = 0.1*clip(dx,10))
                    u = wpool.tile([128, 16, T], f32r, tag="u")
                    nc.vector.tensor_scalar(
                        u[:, :nr, :], ps3[:, :nr, :], bb[:, 2:3], 1.0, add, mn
                    )
                    nc.vector.scalar_tensor_tensor(
                        dst[:, 1 + i0 : 1 + i0 + nr, 1 : 1 + T],
                        u[:, :nr, :],
                        -1.0,
                        src[:, 1 + i0 : 1 + i0 + nr, 1 : 1 + T],
                        mx,
                        add,
                    )
            fin = stA if n_steps % 2 == 0 else stB
            nc.sync.dma_start(out=outp[:, :, :], in_=fin[:, 1 : 1 + HE, :])
    return nc


def make_in_maps(inputs):
    x = np.asarray(inputs["x"], np.float32)
    cond = np.asarray(inputs["cond"]).astype(np.int64)
    embed = np.asarray(inputs["embed"], np.float64)
    film_w = np.asarray(inputs["film_w"], np.float64)
    film_b = np.asarray(inputs["film_b"], np.float64)
    fc1_w = np.asarray(inputs["fc1_w"], np.float64)
    fc1_b = np.asarray(inputs["fc1_b"], np.float64)
    fc2_w = np.asarray(inputs["fc2_w"], np.float64)
    fc2_b = np.asarray(inputs["fc2_b"], np.float64)
    fc3_w = np.asarray(inputs["fc3_w"], np.float64)
    fc3_b = np.asarray(inputs["fc3_b"], np.float64)

    film = embed[cond] @ film_w + film_b  # [B, 256]
    gamma, beta = film[:, :128], film[:, 128:]

    folded = [
        fold_weights(gamma[b], beta[b], fc1_w, fc1_b, fc2_w, fc2_b, fc3_w, fc3_b)
        for b in range(x.shape[0])
    ]
    in_maps = []
    for k in range(8):
        b, half = k // 2, k % 2
        x_ext = x[b, :, 0:HE, :] if half == 0 else x[b, :, W - HE : W, :]
        m = dict(folded[b])
        m["xb"] = shuffle_in(x_ext).reshape(128, NR_TOT, RS)
        in_maps.append(m)
    return in_maps


def assemble_output(results, like):
    y = np.empty_like(like)
    for k in range(8):
        out = unshuffle_out(results[k]["out"])
        b, half = k // 2, k % 2
        if half == 0:
            y[b, :, 0:128, :] = out[:, 0:128, :]
        else:
            y[b, :, 128:256, :] = out[:, 4:HE, :]
    return y


def kernel(**inputs):
    n_steps = int(np.asarray(inputs["n_steps"]))
    x = np.asarray(inputs["x"], np.float32)
    in_maps = make_in_maps(inputs)
    nc = Bacc()
    build_graph(nc, n_steps)
    nc.finalize()
    res = run_bass_kernel_spmd(nc, in_maps, core_ids=list(range(8)))
    return assemble_output(res.results, x)


# revision 22
# speedup vs baseline: 4143.1516x; 4143.1516x over previous
"""Trainium2 Bass kernel for nn_BaseNCA (NCA: 3x3 Sobel + per-pixel MLP, 4 steps).

Sharding: pure data parallel over 8 cores = (batch b, H-half). Each core gets one
batch's top or bottom half of H (128 rows) plus a 4-row halo toward the middle
(1 conv ring per step x 4 steps). No collectives: validity shrinks one row per
step into the halo; the kept 128-row window is exact after 4 steps. The tile
edge that is a true image boundary zero-pads identically to the reference.

Per-core math folding (host side):
  FiLM gamma/beta are step-invariant; with g = gamma, a=|g|, s=sign(g):
    g*relu(p + b1) + beta == s*relu(a*p + a*b1) + beta
  so scale fc1 columns by a, fold s into fc2 rows and beta@fc2_w into the fc2
  bias. The Sobel convs are linear, so fc1 on [x, gx, gy] becomes 9 shifted
  16->128 matmuls with effective kernels Keff[di][dj]; the dx scale 0.1 is
  folded into fc3 (clip bounds become +-1).

Device layout: state [128 partitions = (c + 16*(j%8)), free = (row, t=j//8)]
with one zero pad column on each side of the 32 t-slots (row stride 34) and one
zero guard row above/below the 132 rows. Conv taps dj=-1,0,+1 then live in
adjacent 16-partition blocks, so fc1 is 3 accumulated K=48 matmuls (one per
di, free-offset +-1 row); W-wraparound classes 0/7 split into K=32 + K=16.
fc3 writes dx for all 8 classes of a row block into one PSUM [128, nr*32]
(8 accumulated M=128 matmuls with host-expanded weights), so the clip +
state-update runs on all 128 lanes. Matmuls use float32r (full speed at
N>=256, ~tf32 precision, fp32 accumulate); state stays fp32.
"""

import sys

import numpy as np

sys.path.insert(0, "/opt/trn_rl_repo")

import concourse.bass as bass
import concourse.mybir as mybir
from concourse.bacc import Bacc
from concourse.bass_utils import run_bass_kernel_spmd
from concourse.tile import TileContext

C, HID, W = 16, 128, 256
HE = 132  # extended rows per core (128 kept + 4 halo toward the middle)
T = W // 8  # 32 t-slots per class
RS = T + 2  # row stride incl. one pad col each side
NR_TOT = 1 + HE + 1  # incl. zero guard rows
SX = np.array([[-1.0, 0.0, 1.0], [-2.0, 0.0, 2.0], [-1.0, 0.0, 1.0]], np.float64)
SY = SX.T

ROW_BLOCKS = [(i, min(16, HE - i)) for i in range(0, HE, 16)]  # 8x16 + 1x4

# fc1 matmul plan: for each output class r and row shift di, a list of
# (kind_index, t_offset) matmuls. Each kind is a [128,128] lhsT with the
# Keff blocks embedded at the partition rows of the source classes; rhs is
# always the full 128-partition state (base partition 0, as HW requires).
MM_PLAN = {}
MM_KINDS = []  # list of (di, [(dj, src_class), ...])
for _r in range(8):
    for _di in (-1, 0, 1):
        ent = []
        if 1 <= _r <= 6:
            MM_KINDS.append((_di, [(dj, _r + dj) for dj in (-1, 0, 1)]))
            ent.append((len(MM_KINDS) - 1, 0))
        elif _r == 0:
            MM_KINDS.append((_di, [(0, 0), (1, 1)]))
            ent.append((len(MM_KINDS) - 1, 0))
            MM_KINDS.append((_di, [(-1, 7)]))
            ent.append((len(MM_KINDS) - 1, -1))
        else:
            MM_KINDS.append((_di, [(-1, 6), (0, 7)]))
            ent.append((len(MM_KINDS) - 1, 0))
            MM_KINDS.append((_di, [(1, 0)]))
            ent.append((len(MM_KINDS) - 1, 1))
        MM_PLAN[(_r, _di)] = ent
N_KINDS = len(MM_KINDS)


def fold_weights(gamma, beta, fc1_w, fc1_b, fc2_w, fc2_b, fc3_w, fc3_b):
    """Per-batch folded weights (float64 in, device arrays out)."""
    a = np.abs(gamma)
    s = np.sign(gamma)
    W1x, W1gx, W1gy = fc1_w[0:16], fc1_w[16:32], fc1_w[32:48]

    def keff(di, dj):
        k = SX[di + 1, dj + 1] * W1gx + SY[di + 1, dj + 1] * W1gy
        if di == 0 and dj == 0:
            k = k + W1x
        return k * a[None, :]

    w1 = np.zeros((N_KINDS, 128, 128), np.float64)
    for idx, (di, blocks) in enumerate(MM_KINDS):
        for dj, cls in blocks:
            w1[idx, 16 * cls : 16 * cls + 16, :] = keff(di, dj)
    b1 = a * fc1_b
    w2 = s[:, None] * fc2_w
    b2 = beta @ fc2_w + fc2_b
    w3big = np.zeros((128, 8, 128), np.float64)
    for r in range(8):
        w3big[:, r, 16 * r : 16 * r + 16] = 0.1 * fc3_w
    b3t = np.tile(0.1 * fc3_b, 8)
    f32 = np.float32
    return {
        "w1": np.ascontiguousarray(w1.transpose(1, 0, 2).reshape(128, N_KINDS * 128)).astype(f32),
        "w2": np.ascontiguousarray(w2).astype(f32),
        "w3": np.ascontiguousarray(w3big.reshape(128, 8 * 128)).astype(f32),
        "bb": np.stack([b1, b2, b3t], axis=1).astype(f32),
    }


def shuffle_in(x_ext):
    """[16, 132, 256] -> [128, NR_TOT*RS] blocked layout with zero pads/guards."""
    xb = np.zeros((8, 16, NR_TOT, RS), np.float32)
    for r in range(8):
        xb[r, :, 1 : 1 + HE, 1 : 1 + T] = x_ext[:, :, r::8]
    return xb.reshape(128, NR_TOT * RS)


def unshuffle_out(res):
    """[128, HE*RS] -> [16, 132, 256]."""
    rb = res.reshape(8, 16, HE, RS)
    y = np.empty((16, HE, W), np.float32)
    for r in range(8):
        y[:, :, r::8] = rb[r, :, :, 1 : 1 + T]
    return y


def build_graph(nc, n_steps):
    f32 = mybir.dt.float32
    f32r = mybir.dt.float32r
    relu = mybir.ActivationFunctionType.Relu
    add, mn, mx = mybir.AluOpType.add, mybir.AluOpType.min, mybir.AluOpType.max

    xin = nc.declare_dram_parameter("xb", [128, NR_TOT, RS], f32, isOutput=False)
    w1in = nc.declare_dram_parameter("w1", [128, N_KINDS * 128], f32, isOutput=False)
    w2in = nc.declare_dram_parameter("w2", [128, 128], f32, isOutput=False)
    w3in = nc.declare_dram_parameter("w3", [128, 8 * 128], f32, isOutput=False)
    bbin = nc.declare_dram_parameter("bb", [128, 3], f32, isOutput=False)
    outp = nc.declare_dram_parameter("out", [128, HE, RS], f32r, isOutput=True)

    with TileContext(nc) as tc:
        with (
            tc.tile_pool(name="const", bufs=1) as cpool,
            tc.tile_pool(name="work", bufs=3) as wpool,
            tc.tile_pool(name="ps1", bufs=2, space="PSUM") as ppool1,
            tc.tile_pool(name="ps2", bufs=2, space="PSUM") as ppool2,
            tc.tile_pool(name="ps3", bufs=2, space="PSUM") as ppool3,
        ):
            stA = cpool.tile([128, NR_TOT, RS], f32r, tag="stA")
            stB = cpool.tile([128, NR_TOT, RS], f32r, tag="stB")
            w1 = cpool.tile([128, N_KINDS * 128], f32r, tag="w1")
            w2 = cpool.tile([128, 128], f32r, tag="w2")
            w3 = cpool.tile([128, 8 * 128], f32r, tag="w3")
            bb = cpool.tile([128, 3], f32, tag="bb")

            # Bulk loads go DMA -> staging -> DVE copy so each matmul-feeding
            # tile has ONE producer proc (DVE); direct big DMAs fan out over
            # many DMA queues and blow the per-instruction sync-wait limit.
            stg_x = wpool.tile([128, NR_TOT, RS], f32, tag="stg_x")
            stg_w1 = wpool.tile([128, N_KINDS * 128], f32, tag="stg_w1")
            stg_w2 = wpool.tile([128, 128], f32, tag="stg_w2")
            stg_w3 = wpool.tile([128, 8 * 128], f32, tag="stg_w3")
            nc.sync.dma_start(out=stg_x[:, :, :], in_=xin[:, :, :])
            nc.sync.dma_start(out=stg_w1[:, :], in_=w1in[:, :])
            nc.sync.dma_start(out=stg_w2[:, :], in_=w2in[:, :])
            nc.sync.dma_start(out=stg_w3[:, :], in_=w3in[:, :])
            stg_b = wpool.tile([128, 3], f32, tag="stg_b")
            nc.sync.dma_start(out=stg_b[:, :], in_=bbin[:, :])
            nc.vector.tensor_copy(stA[:, :, :], stg_x[:, :, :])
            nc.vector.tensor_copy(stB[:, :, :], stg_x[:, :, :])
            nc.vector.tensor_copy(bb[:, :], stg_b[:, :])
            nc.vector.tensor_copy(w1[:, :], stg_w1[:, :])
            nc.vector.tensor_copy(w2[:, :], stg_w2[:, :])
            nc.vector.tensor_copy(w3[:, :], stg_w3[:, :])

            for step in range(n_steps):
                src, dst = (stA, stB) if step % 2 == 0 else (stB, stA)
                for i0, nr in ROW_BLOCKS:
                    ps3 = ppool3.tile([128, 16, T], f32, tag="ps3")
                    for r in range(8):
                        ps1 = ppool1.tile([128, 16, T], f32, tag="ps1")
                        # fc1: conv taps via full-K matmuls (zero-embedded lhsT
                        # kinds); rows 1+i0+di .. +nr; t-window shifts for the
                        # W-wraparound classes.
                        mms = []
                        for di in (-1, 0, 1):
                            for idx, toff in MM_PLAN[(r, di)]:
                                mms.append((idx, di, toff))
                        nmm = len(mms)
                        for q, (idx, di, toff) in enumerate(mms):
                            rsl = slice(1 + i0 + di, 1 + i0 + di + nr)
                            csl = slice(1 + toff, 1 + toff + T)
                            nc.tensor.matmul(
                                ps1[:, :nr, :],
                                w1[:, 128 * idx : 128 * (idx + 1)],
                                src[:, rsl, csl],
                                start=(q == 0),
                                stop=(q == nmm - 1),
                            )
                        h1 = wpool.tile([128, 16, T], f32r, tag="h1")
                        nc.scalar.activation(
                            h1[:, :nr, :], ps1[:, :nr, :], relu, bias=bb[:, 0:1], scale=1.0
                        )
                        ps2 = ppool2.tile([128, 16, T], f32, tag="ps2")
                        nc.tensor.matmul(
                            ps2[:, :nr, :],
                            w2[:, :],
                            h1[:, :nr, :],
                            start=True,
                            stop=True,
                        )
                        h2 = wpool.tile([128, 16, T], f32r, tag="h2")
                        nc.vector.tensor_scalar(
                            h2[:, :nr, :], ps2[:, :nr, :], bb[:, 1:2], 0.0, add, mx
                        )
                        nc.tensor.matmul(
                            ps3[:, :nr, :],
                            w3[:, 128 * r : 128 * (r + 1)],
                            h2[:, :nr, :],
                            start=(r == 0),
                            stop=(r == 7),
                        )
                    # u = min(ps3 + b3, 1); dst = max(u, -1) + src  (clip +-1 = 0.1*clip(dx,10))
                    u = wpool.tile([128, 16, T], f32r, tag="u")
                    nc.vector.tensor_scalar(
                        u[:, :nr, :], ps3[:, :nr, :], bb[:, 2:3], 1.0, add, mn
                    )
                    nc.vector.scalar_tensor_tensor(
                        dst[:, 1 + i0 : 1 + i0 + nr, 1 : 1 + T],
                        u[:, :nr, :],
                        -1.0,
                        src[:, 1 + i0 : 1 + i0 + nr, 1 : 1 + T],
                        mx,
                        add,
                    )
            fin = stA if n_steps % 2 == 0 else stB
            nc.sync.dma_start(out=outp[:, :, :], in_=fin[:, 1 : 1 + HE, :])
    return nc


def make_in_maps(inputs):
    x = np.asarray(inputs["x"], np.float32)
    cond = np.asarray(inputs["cond"]).astype(np.int64)
    embed = np.asarray(inputs["embed"], np.float64)
    film_w = np.asarray(inputs["film_w"], np.float64)
    film_b = np.asarray(inputs["film_b"], np.float64)
    fc1_w = np.asarray(inputs["fc1_w"], np.float64)
    fc1_b = np.asarray(inputs["fc1_b"], np.float64)
    fc2_w = np.asarray(inputs["fc2_w"], np.float64)
    fc2_b = np.asarray(inputs["fc2_b"], np.float64)
    fc3_w = np.asarray(inputs["fc3_w"], np.float64)
    fc3_b = np.asarray(inputs["fc3_b"], np.float64)

    film = embed[cond] @ film_w + film_b  # [B, 256]
    gamma, beta = film[:, :128], film[:, 128:]

    folded = [
        fold_weights(gamma[b], beta[b], fc1_w, fc1_b, fc2_w, fc2_b, fc3_w, fc3_b)
        for b in range(x.shape[0])
    ]
    in_maps = []
    for k in range(8):
        b, half = k // 2, k % 2
        x_ext = x[b, :, 0:HE, :] if half == 0 else x[b, :, W - HE : W, :]
        m = dict(folded[b])
        m["xb"] = shuffle_in(x_ext).reshape(128, NR_TOT, RS)
        in_maps.append(m)
    return in_maps


def assemble_output(results, like):
    y = np.empty_like(like)
    for k in range(8):
        out = unshuffle_out(results[k]["out"])
        b, half = k // 2, k % 2
        if half == 0:
            y[b, :, 0:128, :] = out[:, 0:128, :]
        else:
            y[b, :, 128:256, :] = out[:, 4:HE, :]
    return y


def kernel(**inputs):
    n_steps = int(np.asarray(inputs["n_steps"]))
    x = np.asarray(inputs["x"], np.float32)
    in_maps = make_in_maps(inputs)
    nc = Bacc()
    build_graph(nc, n_steps)
    nc.finalize()
    res = run_bass_kernel_spmd(nc, in_maps, core_ids=list(range(8)))
    return assemble_output(res.results, x)
